# revision 25
# baseline (speedup 1.0000x reference)
"""DNC single-step kernel for Trainium2, 8-core data-parallel over batch.

Self-contained: hardcodes shapes for nn_DNC (B=64, V=64, H=512, O=64,
N=1024, Wd=64, R=4, XI=471). Sharding: 8 samples per core; weights
replicated (W_ih/W_hh pre-transposed + bf16-cast on host as layout prep).
"""
import sys
sys.path.insert(0, "/opt/trn_rl_repo")

import numpy as np
import ml_dtypes

import concourse.bass as bass
import concourse.tile as tile
import concourse.mybir as mybir
from concourse import bacc
from concourse.bass_utils import run_bass_kernel_spmd

F32 = mybir.dt.float32
BF16 = mybir.dt.bfloat16
AF = mybir.ActivationFunctionType
OP = mybir.AluOpType
AXX = mybir.AxisListType.X

B, V, H, O, N, Wd, R = 64, 64, 512, 64, 1024, 64, 4
XI = 471
EPS = 1e-6
NCORES = 8
BS = B // NCORES          # samples per core
NB = N // 128             # 128-row blocks of memory
P = 128

OFF_RSTR, OFF_WK, OFF_WSTR = 256, 260, 324
OFF_ER, OFF_WV, OFF_FG, OFF_AG, OFF_WG, OFF_RM = 325, 389, 453, 457, 458, 459


def dnc_core(ctx, tc, I, Ou):
    nc = tc.nc

    consts = ctx.enter_context(tc.tile_pool(name="consts", bufs=1))
    ident = consts.tile([P, P], F32, name="ident")
    identb = consts.tile([P, P], BF16, name="identb")
    maskoff = consts.tile([P, P], BF16, name="maskoff")
    ones64f = consts.tile([Wd, 1], F32, name="ones64f")
    ones_row = consts.tile([1, N], F32, name="ones_row")
    nc.vector.memset(ones_row[:], 1.0)
    ones64b = consts.tile([Wd, R], BF16, name="ones64b")
    nc.sync.dma_start(ident[:], I["ident"])
    nc.sync.dma_start(identb[:], I["identb"])
    nc.sync.dma_start(maskoff[:], I["maskoff"])
    nc.vector.memset(ones64f[:], 1.0)
    nc.vector.memset(ones64b[:], 1.0)

    # ---------------- weights ----------------
    wpool = ctx.enter_context(tc.tile_pool(name="weights", bufs=1))
    wt_ih, wt_hh, w_xi, w_v, w_rd = [], [], [], [], []
    for kb, rows in enumerate([128, 128, 64]):
        t = wpool.tile([rows, 4 * H], BF16, name=f"wtih{kb}")
        nc.sync.dma_start(t[:], I["W_ihT"][kb * 128:kb * 128 + rows, :])
        wt_ih.append(t)
    for kb in range(4):
        t = wpool.tile([128, 4 * H], BF16, name=f"wthh{kb}")
        nc.sync.dma_start(t[:], I["W_hhT"][kb * 128:(kb + 1) * 128, :])
        wt_hh.append(t)
    for kb in range(4):
        t = wpool.tile([128, XI], F32, name=f"wxi{kb}")
        nc.sync.dma_start(t[:], I["W_xi"][kb * 128:(kb + 1) * 128, :])
        w_xi.append(t)
    for kb in range(4):
        t = wpool.tile([128, O], F32, name=f"wv{kb}")
        nc.sync.dma_start(t[:], I["W_v"][kb * 128:(kb + 1) * 128, :])
        w_v.append(t)
    for kb in range(4):
        t = wpool.tile([Wd, O], F32, name=f"wrd{kb}")
        nc.sync.dma_start(t[:], I["W_read"][kb * Wd:(kb + 1) * Wd, :])
        w_rd.append(t)

    rows_p = ctx.enter_context(tc.tile_pool(name="rows", bufs=1))

    # M tiles: per sample (128, NB*Wd) f32, free=(blk, w)
    mpool = ctx.enter_context(tc.tile_pool(name="mpool", bufs=1))
    m_s = []
    for s in range(BS):
        t = mpool.tile([P, NB * Wd], F32, name=f"m_s{s}")
        nc.sync.dma_start(t[:].rearrange("p (nb w) -> p nb w", nb=NB),
                          I["M"][s].rearrange("(nb p) w -> p nb w", p=P))
        m_s.append(t)
    # wr columns per sample (128, NB*R) bf16, free=(blk, r)
    wrb_s = []
    for s in range(BS):
        t = mpool.tile([P, NB * R], BF16, name=f"wrb{s}")
        nc.gpsimd.dma_start(t[:].rearrange("p (nb r) -> p nb r", nb=NB),
                            I["wr"][s].rearrange("(nb p) r -> p nb r", p=P))
        wrb_s.append(t)

    # =====================================================================
    # Phase A: controller LSTM + interface decode
    # =====================================================================
    with tc.tile_pool(name="pa_sb", bufs=1) as pa, \
         tc.tile_pool(name="pa_ps", bufs=2, space="PSUM") as pps:
        x_t = pa.tile([BS, V], F32, name="x_t")
        h0_t = pa.tile([BS, H], F32, name="h0_t")
        c0_t = pa.tile([BS, H], F32, name="c0_t")
        rv_t = pa.tile([BS, Wd * R], F32, name="rv_t")
        bias_t = pa.tile([BS, 4 * H], F32, name="bias_t")
        nc.sync.dma_start(x_t[:], I["x"])
        nc.sync.dma_start(h0_t[:], I["h0"])
        nc.sync.dma_start(c0_t[:], I["c0"])
        nc.sync.dma_start(rv_t[:], I["rv"].rearrange("b w r -> b (w r)"))
        nc.sync.dma_start(bias_t[:], I["b_lstm"][None, :].to_broadcast((BS, 4 * H)))

        inpT = pa.tile([P, 3 * BS], BF16, name="inpT")
        ps_x = pps.tile([P, BS], F32, name="ps_x", tag="pa")
        nc.tensor.transpose(ps_x[:V, :], x_t[:], ident[:BS, :BS])
        nc.scalar.copy(inpT[0:V, 0:BS], ps_x[:V, :])
        rv3 = rv_t[:].rearrange("b (w r) -> b r w", r=R)
        for r in range(R):
            ps_r = pps.tile([P, BS], F32, name=f"ps_rv{r}", tag="pa")
            nc.tensor.transpose(ps_r[:Wd, :], rv3[:, r, :], ident[:BS, :BS])
            row0 = V + r * Wd
            kb, off = row0 // 128, row0 % 128
            nc.scalar.copy(inpT[off:off + Wd, kb * BS:(kb + 1) * BS], ps_r[:Wd, :])
        h0T = pa.tile([P, 4 * BS], BF16, name="h0T")
        for kb in range(4):
            ps_h = pps.tile([P, BS], F32, name=f"ps_h{kb}", tag="pa")
            nc.tensor.transpose(ps_h[:], h0_t[:, kb * 128:(kb + 1) * 128], ident[:BS, :BS])
            nc.scalar.copy(h0T[:, kb * BS:(kb + 1) * BS], ps_h[:])

        gates = pa.tile([BS, 4 * H], F32, name="gates")
        for c in range(4):
            gps = pps.tile([BS, 512], F32, name=f"gps{c}", tag="gps")
            fs = slice(c * 512, (c + 1) * 512)
            for kb, rows in enumerate([128, 128, 64]):
                nc.tensor.matmul(gps[:], inpT[:rows, kb * BS:(kb + 1) * BS],
                                 wt_ih[kb][:rows, fs], start=(kb == 0), stop=False)
            for kb in range(4):
                nc.tensor.matmul(gps[:], h0T[:, kb * BS:(kb + 1) * BS],
                                 wt_hh[kb][:, fs], start=False, stop=(kb == 3))
            nc.vector.tensor_add(gates[:, fs], gps[:], bias_t[:, fs])

        si = pa.tile([BS, H], F32, name="si")
        sf = pa.tile([BS, H], F32, name="sf")
        tg = pa.tile([BS, H], F32, name="tg")
        so = pa.tile([BS, H], F32, name="so")
        nc.scalar.activation(si[:], gates[:, 0:H], AF.Sigmoid)
        nc.scalar.activation(sf[:], gates[:, H:2 * H], AF.Sigmoid)
        nc.scalar.activation(tg[:], gates[:, 2 * H:3 * H], AF.Tanh)
        nc.scalar.activation(so[:], gates[:, 3 * H:4 * H], AF.Sigmoid)
        c_new = pa.tile([BS, H], F32, name="c_new")
        nc.vector.tensor_mul(c_new[:], sf[:], c0_t[:])
        nc.vector.tensor_mul(si[:], si[:], tg[:])
        nc.vector.tensor_add(c_new[:], c_new[:], si[:])
        nc.sync.dma_start(Ou["c"], c_new[:])
        h_new = pa.tile([BS, H], F32, name="h_new")
        nc.scalar.activation(h_new[:], c_new[:], AF.Tanh)
        nc.vector.tensor_mul(h_new[:], so[:], h_new[:])
        nc.sync.dma_start(Ou["h"], h_new[:])

        hT = pa.tile([P, 4 * BS], F32, name="hT")
        for kb in range(4):
            ps_h2 = pps.tile([P, BS], F32, name=f"ps_hh{kb}", tag="pa")
            nc.tensor.transpose(ps_h2[:], h_new[:, kb * 128:(kb + 1) * 128], ident[:BS, :BS])
            nc.scalar.copy(hT[:, kb * BS:(kb + 1) * BS], ps_h2[:])

        # vu^T (64, BS) -> SBUF (final y = vu + read part, added at the end)
        vu_ps = pps.tile([O, BS], F32, name="vu_ps", tag="gps")
        for kb in range(4):
            nc.tensor.matmul(vu_ps[:], w_v[kb][:], hT[:, kb * BS:(kb + 1) * BS],
                             start=(kb == 0), stop=(kb == 3))
        vu_sb = rows_p.tile([O, BS], F32, name="vu_sb")
        nc.scalar.copy(vu_sb[:], vu_ps[:])

        # xi^T -> xi_rows (BS, 512)
        xi_rows = rows_p.tile([BS, 512], F32, name="xi_rows")
        for xb in range(4):
            rows = min(128, XI - xb * 128)
            xps = pps.tile([P, BS], F32, name=f"xps{xb}", tag="xps")
            for kb in range(4):
                nc.tensor.matmul(xps[:rows, :], w_xi[kb][:, xb * 128:xb * 128 + rows],
                                 hT[:, kb * BS:(kb + 1) * BS],
                                 start=(kb == 0), stop=(kb == 3))
            xsb = pa.tile([P, BS], F32, name=f"xsb{xb}", tag="xsb")
            nc.scalar.copy(xsb[:rows, :], xps[:rows, :])
            xrp = pps.tile([BS, P], F32, name=f"xrp{xb}", tag="xps2")
            nc.tensor.transpose(xrp[:, :rows], xsb[:rows, :], ident[:rows, :rows])
            nc.scalar.copy(xi_rows[:, xb * 128:xb * 128 + rows], xrp[:, :rows])

        # write_key^T (64, BS) at partition base 0
        wkT = rows_p.tile([Wd, BS], F32, name="wkT")
        wkps = pps.tile([Wd, BS], F32, name="wkps", tag="xps")
        for kb in range(4):
            nc.tensor.matmul(wkps[:], w_xi[kb][:, OFF_WK:OFF_WK + Wd],
                             hT[:, kb * BS:(kb + 1) * BS], start=(kb == 0), stop=(kb == 3))
        nc.scalar.copy(wkT[:], wkps[:])

        er_row = rows_p.tile([BS, Wd], F32, name="er_row")
        nc.scalar.activation(er_row[:], xi_rows[:, OFF_ER:OFF_ER + Wd], AF.Sigmoid)
        ner_row = rows_p.tile([BS, Wd], F32, name="ner_row")
        nc.vector.tensor_scalar(out=ner_row[:], in0=er_row[:], scalar1=-1.0,
                                scalar2=None, op0=OP.mult)
        wv_row = rows_p.tile([BS, Wd], F32, name="wv_row")
        nc.vector.tensor_copy(wv_row[:], xi_rows[:, OFF_WV:OFF_WV + Wd])
        fg_row = rows_p.tile([BS, R], F32, name="fg_row")
        nc.scalar.activation(fg_row[:], xi_rows[:, OFF_FG:OFF_FG + R], AF.Sigmoid)
        ag_row = rows_p.tile([BS, 1], F32, name="ag_row")
        nc.scalar.activation(ag_row[:], xi_rows[:, OFF_AG:OFF_AG + 1], AF.Sigmoid)
        omag_row = rows_p.tile([BS, 1], F32, name="omag_row")
        nc.vector.tensor_scalar(out=omag_row[:], in0=ag_row[:], scalar1=-1.0,
                                scalar2=1.0, op0=OP.mult, op1=OP.add)
        wg_row = rows_p.tile([BS, 1], F32, name="wg_row")
        nc.scalar.activation(wg_row[:], xi_rows[:, OFF_WG:OFF_WG + 1], AF.Sigmoid)

        def oneplus(dst, src_sl, n):
            tmp = pa.tile([BS, n], F32, name="op_tmp", tag="op_tmp")
            nc.scalar.activation(tmp[:], src_sl, AF.Exp)
            nc.scalar.activation(tmp[:], tmp[:], AF.Ln, bias=1.0)
            nc.vector.tensor_scalar(out=dst, in0=tmp[:], scalar1=1.0, scalar2=None,
                                    op0=OP.add)
        rs_row = rows_p.tile([BS, R], F32, name="rs_row")
        oneplus(rs_row[:], xi_rows[:, OFF_RSTR:OFF_RSTR + R], R)
        ws_row = rows_p.tile([BS, 1], F32, name="ws_row")
        oneplus(ws_row[:], xi_rows[:, OFF_WSTR:OFF_WSTR + 1], 1)

        # read modes softmax (over the 3 modes)
        mx = pa.tile([BS, R], F32, name="rm_mx")
        nc.vector.tensor_tensor(out=mx[:], in0=xi_rows[:, OFF_RM:OFF_RM + R],
                                in1=xi_rows[:, OFF_RM + R:OFF_RM + 2 * R], op=OP.max)
        nc.vector.tensor_tensor(out=mx[:], in0=mx[:],
                                in1=xi_rows[:, OFF_RM + 2 * R:OFF_RM + 3 * R], op=OP.max)
        sm = pa.tile([BS, R], F32, name="rm_sm")
        e_m = []
        for m in range(3):
            e = pa.tile([BS, R], F32, name=f"rm_e{m}")
            nc.vector.tensor_tensor(out=e[:], in0=xi_rows[:, OFF_RM + m * R:OFF_RM + (m + 1) * R],
                                    in1=mx[:], op=OP.subtract)
            nc.scalar.activation(e[:], e[:], AF.Exp)
            e_m.append(e)
            if m == 0:
                nc.vector.tensor_copy(sm[:], e[:])
            else:
                nc.vector.tensor_add(sm[:], sm[:], e[:])
        nc.vector.reciprocal(sm[:], sm[:])
        pi_cols = []
        for m in range(3):
            pi = pa.tile([BS, R], F32, name=f"pi_row{m}")
            nc.vector.tensor_mul(pi[:], e_m[m][:], sm[:])
            pps_m = pps.tile([R, BS], F32, name=f"pi_ps{m}", tag="pa")
            nc.tensor.transpose(pps_m[:], pi[:], ident[:BS, :BS])
            pc = rows_p.tile([R, BS], F32, name=f"pi_col{m}")
            nc.scalar.copy(pc[:], pps_m[:])
            pi_cols.append(pc)

        # read strengths / rkn2 / kwn2
        rsT = rows_p.tile([R, BS], F32, name="rsT")
        rs_ps = pps.tile([R, BS], F32, name="rs_ps", tag="pa")
        nc.tensor.transpose(rs_ps[:], rs_row[:], ident[:BS, :BS])
        nc.scalar.copy(rsT[:], rs_ps[:])

        rkn2 = rows_p.tile([BS, R], F32, name="rkn2")
        xisq = pa.tile([BS, Wd * R], F32, name="xisq")
        nc.vector.tensor_mul(xisq[:], xi_rows[:, 0:Wd * R], xi_rows[:, 0:Wd * R])
        xisq4 = xisq[:].rearrange("b (w r) -> b r w", r=R)
        for r in range(R):
            nc.vector.reduce_sum(rkn2[:, r:r + 1], xisq4[:, r, :], axis=AXX)
        rk2T = rows_p.tile([R, BS], F32, name="rk2T")
        rk2ps = pps.tile([R, BS], F32, name="rk2ps", tag="pa")
        nc.tensor.transpose(rk2ps[:], rkn2[:], ident[:BS, :BS])
        nc.scalar.copy(rk2T[:], rk2ps[:])

        kwn2 = rows_p.tile([BS, 1], F32, name="kwn2")
        wksq = pa.tile([BS, Wd], F32, name="wksq")
        nc.vector.tensor_mul(wksq[:], xi_rows[:, OFF_WK:OFF_WK + Wd],
                             xi_rows[:, OFF_WK:OFF_WK + Wd])
        nc.vector.reduce_sum(kwn2[:], wksq[:], axis=AXX)

        # read keys transposed per head: rkT[r] (Wd, BS) bf16
        rkT = []
        xirk = xi_rows[:, 0:Wd * R].rearrange("b (w r) -> b r w", r=R)
        for r in range(R):
            rps = pps.tile([Wd, BS], F32, name=f"rk_ps{r}", tag="pa")
            nc.tensor.transpose(rps[:], xirk[:, r, :], ident[:BS, :BS])
            rk_t = rows_p.tile([Wd, BS], BF16, name=f"rkT{r}")
            nc.scalar.copy(rk_t[:], rps[:])
            rkT.append(rk_t)

    # =====================================================================
    # Phase B: u, allocation
    # =====================================================================
    pb_ctx = tc.tile_pool(name="pb_sb", bufs=1)
    pb = pb_ctx.__enter__()
    usage_r = pb.tile([BS, N], F32, name="usage_r")
    wwin_r = pb.tile([BS, N], F32, name="wwin_r")
    prec_r = pb.tile([BS, N], F32, name="prec_r")
    wr_rows = pb.tile([BS, N * R], F32, name="wr_rows")
    nc.sync.dma_start(usage_r[:], I["usage"])
    nc.sync.dma_start(wwin_r[:], I["ww"])
    nc.sync.dma_start(prec_r[:], I["prec"])
    nc.sync.dma_start(wr_rows[:], I["wr"].rearrange("b n r -> b (n r)"))

    nfg = pb.tile([BS, R], F32, name="nfg")
    nc.vector.tensor_scalar(out=nfg[:], in0=fg_row[:], scalar1=-1.0, scalar2=None,
                            op0=OP.mult)
    wr4 = wr_rows[:].rearrange("b (n r) -> b r n", r=R)
    psi = pb.tile([BS, N], F32, name="psi")
    tpr = pb.tile([BS, N], F32, name="tpr")
    for r in range(R):
        dst = psi if r == 0 else tpr
        nc.vector.tensor_scalar(out=dst[:], in0=wr4[:, r, :], scalar1=nfg[:, r:r + 1],
                                scalar2=1.0, op0=OP.mult, op1=OP.add)
        if r > 0:
            nc.vector.tensor_mul(psi[:], psi[:], tpr[:])

    u_row = pb.tile([BS, N], F32, name="u_row")
    t_a = pb.tile([BS, N], F32, name="t_a")
    t_b = pb.tile([BS, N], F32, name="t_b")
    nc.vector.tensor_add(t_a[:], usage_r[:], wwin_r[:])
    nc.vector.tensor_mul(t_b[:], usage_r[:], wwin_r[:])
    nc.vector.tensor_sub(t_a[:], t_a[:], t_b[:])
    nc.vector.tensor_mul(u_row[:], t_a[:], psi[:])
    nc.sync.dma_start(Ou["u"], u_row[:])

    # allocation: exact bottom-8 of u
    nu = pb.tile([BS, N], F32, name="nu")
    nc.vector.tensor_scalar(out=nu[:], in0=u_row[:], scalar1=-1.0, scalar2=None, op0=OP.mult)
    m8 = pb.tile([BS, 8], F32, name="m8")
    nc.vector.max(out=m8[:], in_=nu[:])
    mvals = pb.tile([BS, 8], F32, name="mvals")
    nc.vector.tensor_scalar(out=mvals[:], in0=m8[:], scalar1=-1.0, scalar2=None, op0=OP.mult)
    incl = pb.tile([BS, 8], F32, name="incl")
    nc.vector.tensor_tensor_scan(out=incl[:], data0=mvals[:], data1=mvals[:],
                                 initial=1.0, op0=OP.mult, op1=OP.bypass)
    excl = pb.tile([BS, 8], F32, name="excl")
    nc.vector.memset(excl[:, 0:1], 1.0)
    nc.vector.tensor_copy(excl[:, 1:8], incl[:, 0:7])
    av = pb.tile([BS, 8], F32, name="av")
    nc.vector.tensor_scalar(out=av[:], in0=mvals[:], scalar1=-1.0, scalar2=1.0,
                            op0=OP.mult, op1=OP.add)
    nc.vector.tensor_mul(av[:], av[:], excl[:])
    a_row = pb.tile([BS, N], F32, name="a_row")
    a_tmp = pb.tile([BS, N], F32, name="a_tmp")
    nc.vector.memset(a_tmp[:], 0.0)
    msk = pb.tile([BS, N], F32, name="msk")
    cur, nxt = a_tmp, a_row
    for k in range(8):
        nc.vector.tensor_scalar(out=msk[:], in0=u_row[:], scalar1=mvals[:, k:k + 1],
                                scalar2=None, op0=OP.is_equal)
        nc.vector.scalar_tensor_tensor(out=nxt[:], in0=msk[:], scalar=av[:, k:k + 1],
                                       in1=cur[:], op0=OP.mult, op1=OP.add)
        cur, nxt = nxt, cur
    if cur is not a_row:
        nc.vector.tensor_copy(a_row[:], cur[:])

    # =====================================================================
    # Phase B-M: M^T, sim_w dot, Mn2 (per sample, row results via tiny DMA)
    # =====================================================================
    simw_row = pb.tile([BS, N], F32, name="simw_row")
    mn2_rows = pb.tile([BS, N], F32, name="mn2_rows")
    with tc.tile_pool(name="pm_sb", bufs=1) as pm, \
         tc.tile_pool(name="pm_ps", bufs=1, space="PSUM") as pmps, \
         tc.tile_pool(name="pm_ps2", bufs=1, space="PSUM") as pmps2:
        for s in range(BS):
            mt_ps = pmps.tile([Wd, N], F32, name="mt_ps", tag="mt")
            for blk in range(NB):
                nc.tensor.transpose(mt_ps[:, blk * 128:(blk + 1) * 128],
                                    m_s[s][:, blk * Wd:(blk + 1) * Wd], ident[:])
            mt_sb = pm.tile([Wd, N], F32, name="mt_sb", tag="mtsb")
            nc.scalar.copy(mt_sb[:], mt_ps[:])
            mtsq = pm.tile([Wd, N], F32, name="mtsq", tag="mtsq")
            nc.vector.tensor_mul(mtsq[:], mt_sb[:], mt_sb[:])
            dot_ps = pmps2.tile([1, N], F32, name="dot_ps", tag="dps")
            nn_ps = pmps2.tile([1, N], F32, name="nn_ps", tag="nps")
            for ch in range(2):
                cs = slice(ch * 512, (ch + 1) * 512)
                nc.tensor.matmul(dot_ps[:, cs], wkT[:, s:s + 1], mt_sb[:, cs],
                                 start=True, stop=True)
                nc.tensor.matmul(nn_ps[:, cs], ones64f[:], mtsq[:, cs],
                                 start=True, stop=True)
            dot_sb = pm.tile([1, N], F32, name="dot_sb", tag="dsb")
            nn_sb = pm.tile([1, N], F32, name="nn_sb", tag="nsb")
            nc.scalar.copy(dot_sb[:], dot_ps[:])
            nc.scalar.copy(nn_sb[:], nn_ps[:])
            nc.sync.dma_start(simw_row[s:s + 1, :], dot_sb[:])
            nc.sync.dma_start(mn2_rows[s:s + 1, :], nn_sb[:])

    # =====================================================================
    # Phase B3: cw, ww_n, prec_n, column/broadcast prep
    # =====================================================================
    # cw = softmax(ws * dot / (sqrt(mn2*kwn2) + EPS))
    den = pb.tile([BS, N], F32, name="den")
    nc.vector.tensor_scalar(out=den[:], in0=mn2_rows[:], scalar1=kwn2[:], scalar2=None,
                            op0=OP.mult)
    nc.scalar.activation(den[:], den[:], AF.Sqrt)
    nc.vector.tensor_scalar(out=den[:], in0=den[:], scalar1=EPS, scalar2=None, op0=OP.add)
    nc.vector.reciprocal(den[:], den[:])
    cw_row = pb.tile([BS, N], F32, name="cw_row")
    nc.vector.tensor_mul(cw_row[:], simw_row[:], den[:])
    nc.vector.tensor_scalar(out=cw_row[:], in0=cw_row[:], scalar1=ws_row[:], scalar2=None,
                            op0=OP.mult)
    smx = pb.tile([BS, 1], F32, name="smx")
    nc.vector.reduce_max(smx[:], cw_row[:], axis=AXX)
    nsmx = pb.tile([BS, 1], F32, name="nsmx")
    nc.vector.tensor_scalar(out=nsmx[:], in0=smx[:], scalar1=-1.0, scalar2=None, op0=OP.mult)
    nc.scalar.activation(cw_row[:], cw_row[:], AF.Exp, bias=nsmx[:])
    ssum = pb.tile([BS, 1], F32, name="ssum")
    nc.vector.reduce_sum(ssum[:], cw_row[:], axis=AXX)
    nc.vector.reciprocal(ssum[:], ssum[:])
    nc.vector.tensor_scalar(out=cw_row[:], in0=cw_row[:], scalar1=ssum[:], scalar2=None,
                            op0=OP.mult)

    # ww_n = wg * (ag*a + (1-ag)*cw)
    wwn_row = rows_p.tile([BS, N], F32, name="wwn_row")
    t1 = pb.tile([BS, N], F32, name="t1")
    nc.vector.tensor_scalar(out=t1[:], in0=a_row[:], scalar1=ag_row[:], scalar2=None,
                            op0=OP.mult)
    nc.vector.scalar_tensor_tensor(out=wwn_row[:], in0=cw_row[:], scalar=omag_row[:],
                                   in1=t1[:], op0=OP.mult, op1=OP.add)
    nc.vector.tensor_scalar(out=wwn_row[:], in0=wwn_row[:], scalar1=wg_row[:], scalar2=None,
                            op0=OP.mult)
    nc.sync.dma_start(Ou["ww_n"], wwn_row[:])

    # prec_n = (1 - sum(ww_n)) * prec + ww_n
    swn = pb.tile([BS, 1], F32, name="swn")
    nc.vector.reduce_sum(swn[:], wwn_row[:], axis=AXX)
    nc.vector.tensor_scalar(out=swn[:], in0=swn[:], scalar1=-1.0, scalar2=1.0,
                            op0=OP.mult, op1=OP.add)
    precn_row = pb.tile([BS, N], F32, name="precn_row")
    nc.vector.scalar_tensor_tensor(out=precn_row[:], in0=prec_r[:], scalar=swn[:],
                                   in1=wwn_row[:], op0=OP.mult, op1=OP.add)
    nc.sync.dma_start(Ou["prec_n"], precn_row[:])

    # column layouts: wwn_cols (128, NB*BS) free=(blk, s); omw = 1-wwn
    wwn_cols = rows_p.tile([P, NB * BS], F32, name="wwn_cols")
    with tc.tile_pool(name="pc_ps", bufs=1, space="PSUM") as pcps:
        wc_ps = pcps.tile([P, NB * BS], F32, name="wc_ps")
        for blk in range(NB):
            nc.tensor.transpose(wc_ps[:, blk * BS:(blk + 1) * BS],
                                wwn_row[:, blk * 128:(blk + 1) * 128], ident[:BS, :BS])
        nc.scalar.copy(wwn_cols[:], wc_ps[:])
    omw_cols = rows_p.tile([P, NB * BS], F32, name="omw_cols")
    nc.vector.tensor_scalar(out=omw_cols[:], in0=wwn_cols[:], scalar1=-1.0, scalar2=1.0,
                            op0=OP.mult, op1=OP.add)

    negw_bf = rows_p.tile([BS, N], BF16, name="negw_bf")
    nc.vector.tensor_scalar(out=negw_bf[:], in0=wwn_row[:], scalar1=-1.0, scalar2=None,
                            op0=OP.mult)
    prec_bf = rows_p.tile([BS, N], BF16, name="prec_bf")
    nc.vector.tensor_copy(prec_bf[:], prec_r[:])
    nc.sync.dma_start(I["sc_negw"], negw_bf[:])
    nc.sync.dma_start(I["sc_prec"], prec_bf[:])


    # =====================================================================
    # Phase B2: M_n, M_n^T, Mnn2, sim_r (per sample)
    # =====================================================================
    mnew_s = []
    simr32 = rows_p.tile([4 * BS, N], F32, name="simr32")
    mnb32 = rows_p.tile([4 * BS, N], F32, name="mnb32")
    with tc.tile_pool(name="p2_sb", bufs=1) as p2, \
         tc.tile_pool(name="p2_ps", bufs=1, space="PSUM") as p2ps, \
         tc.tile_pool(name="p2_ps2", bufs=1, space="PSUM") as p2ps2:
        for s in range(BS):
            # per-sample base-0 rows (tiny SBUF->SBUF DMAs)
            lt2 = p2.tile([2, N], F32, name="lt2", tag="lt2")
            nc.sync.dma_start(lt2[0:1, :], wwn_row[s:s + 1, :])
            nc.sync.dma_start(lt2[1:2, :], ones_row[:])
            rh2 = p2.tile([2, Wd], F32, name="rh2", tag="rh2")
            nc.sync.dma_start(rh2[0:1, :], ner_row[s:s + 1, :])
            nc.sync.dma_start(rh2[1:2, :], ones_row[:, :Wd])
            wv0t = p2.tile([1, Wd], F32, name="wv0t", tag="wv0t")
            nc.sync.dma_start(wv0t[:], wv_row[s:s + 1, :])
            # F = 1 - wwn_i * erase_w ; V1 = wwn_i * wv_w   (free=(blk,w))
            f_ps = p2ps.tile([P, NB * Wd], F32, name="f_ps2", tag="fps2")
            v_ps = p2ps.tile([P, NB * Wd], F32, name="v_ps2", tag="vps2")
            for blk in range(NB):
                bsl = slice(blk * Wd, (blk + 1) * Wd)
                ksl = slice(blk * 128, (blk + 1) * 128)
                nc.tensor.matmul(f_ps[:, bsl], lt2[:, ksl], rh2[:],
                                 start=True, stop=True)
                nc.tensor.matmul(v_ps[:, bsl], lt2[0:1, ksl], wv0t[:],
                                 start=True, stop=True)
            mn_t = p2.tile([P, NB * Wd], F32, name="mn_t", tag="mnt")
            nc.vector.tensor_mul(mn_t[:], m_s[s][:], f_ps[:])
            mnew = mpool.tile([P, NB * Wd], BF16, name=f"mnew{s}")
            nc.vector.tensor_add(mnew[:], mn_t[:], v_ps[:])
            mnew_s.append(mnew)
            nc.gpsimd.dma_start(Ou["M_n"][s].rearrange("(nb p) w -> p nb w", p=P),
                                mnew[:].rearrange("p (nb w) -> p nb w", nb=NB))
            # M_n^T (64, N) bf16
            mnt_ps = p2ps.tile([Wd, N], BF16, name="mnt_ps", tag="mntps")
            for blk in range(NB):
                nc.tensor.transpose(mnt_ps[:, blk * 128:(blk + 1) * 128],
                                    mnew[:, blk * Wd:(blk + 1) * Wd], identb[:])
            mnt_sb = p2.tile([Wd, N], BF16, name="mnt_sb", tag="mntsb")
            nc.scalar.copy(mnt_sb[:], mnt_ps[:])
            mntsq = p2.tile([Wd, N], BF16, name="mntsq", tag="mntsq")
            nc.vector.tensor_mul(mntsq[:], mnt_sb[:], mnt_sb[:])
            # sim_r dot (4, N) + mnn2 (1, N)
            rk_s = p2.tile([Wd, R], BF16, name="rk_s", tag="rks")
            for r in range(R):
                nc.vector.tensor_copy(rk_s[:, r:r + 1], rkT[r][:, s:s + 1])
            sr_ps = p2ps2.tile([R, N], F32, name="sr_ps", tag="srps")
            n2_ps = p2ps2.tile([R, N], F32, name="n2_ps", tag="n2ps")
            for ch in range(2):
                cs = slice(ch * 512, (ch + 1) * 512)
                nc.tensor.matmul(sr_ps[:, cs], rk_s[:], mnt_sb[:, cs], start=True, stop=True)
                nc.tensor.matmul(n2_ps[:, cs], ones64b[:], mntsq[:, cs], start=True, stop=True)
            sr_sb = p2.tile([R, N], F32, name="sr_sb", tag="srsb")
            nc.scalar.copy(sr_sb[:], sr_ps[:])
            n2_sb = p2.tile([R, N], F32, name="n2_sb", tag="n2sb")
            nc.scalar.copy(n2_sb[:], n2_ps[:])
            nc.sync.dma_start(simr32[4 * s:4 * s + 4, :], sr_sb[:])
            nc.sync.dma_start(mnb32[4 * s:4 * s + 4, :], n2_sb[:])

    # =====================================================================
    # Phase B4: cr (batched softmax over N), scaled by pi_c
    # =====================================================================
    rk2_32 = rows_p.tile([4 * BS, 1], F32, name="rk2_32")
    rs32 = rows_p.tile([4 * BS, 1], F32, name="rs32")
    pib32 = rows_p.tile([4 * BS, 1], F32, name="pib32")
    pic32 = rows_p.tile([4 * BS, 1], F32, name="pic32")
    pif32 = rows_p.tile([4 * BS, 1], F32, name="pif32")
    for s in range(BS):
        nc.sync.dma_start(rk2_32[4 * s:4 * s + 4, :], rk2T[:, s:s + 1])
        nc.sync.dma_start(rs32[4 * s:4 * s + 4, :], rsT[:, s:s + 1])
        nc.sync.dma_start(pib32[4 * s:4 * s + 4, :], pi_cols[0][:, s:s + 1])
        nc.sync.dma_start(pic32[4 * s:4 * s + 4, :], pi_cols[1][:, s:s + 1])
        nc.sync.dma_start(pif32[4 * s:4 * s + 4, :], pi_cols[2][:, s:s + 1])

    den32 = pb.tile([4 * BS, N], F32, name="den32")
    nc.vector.tensor_scalar(out=den32[:], in0=mnb32[:], scalar1=rk2_32[:], scalar2=None,
                            op0=OP.mult)
    nc.scalar.activation(den32[:], den32[:], AF.Sqrt)
    nc.vector.tensor_scalar(out=den32[:], in0=den32[:], scalar1=EPS, scalar2=None, op0=OP.add)
    nc.vector.reciprocal(den32[:], den32[:])
    cr32 = rows_p.tile([4 * BS, N], F32, name="cr32")
    nc.vector.tensor_mul(cr32[:], simr32[:], den32[:])
    nc.vector.tensor_scalar(out=cr32[:], in0=cr32[:], scalar1=rs32[:], scalar2=None,
                            op0=OP.mult)
    smx32 = pb.tile([4 * BS, 1], F32, name="smx32")
    nc.vector.reduce_max(smx32[:], cr32[:], axis=AXX)
    nc.vector.tensor_scalar(out=smx32[:], in0=smx32[:], scalar1=-1.0, scalar2=None, op0=OP.mult)
    nc.scalar.activation(cr32[:], cr32[:], AF.Exp, bias=smx32[:])
    ssum32 = pb.tile([4 * BS, 1], F32, name="ssum32")
    nc.vector.reduce_sum(ssum32[:], cr32[:], axis=AXX)
    nc.vector.reciprocal(ssum32[:], ssum32[:])
    nc.vector.tensor_scalar(out=cr32[:], in0=cr32[:], scalar1=ssum32[:], scalar2=None,
                            op0=OP.mult)
    # scale by pi_c now; redistribute per-sample (base 0)
    nc.vector.tensor_scalar(out=cr32[:], in0=cr32[:], scalar1=pic32[:], scalar2=None,
                            op0=OP.mult)
    pb_ctx.__exit__(None, None, None)

    # =====================================================================
    # Phase C: link -> L -> (L out, bwd, L^T, fwd) per sample
    # =====================================================================
    rvnT = [rows_p.tile([Wd, BS], F32, name=f"rvnT{r}") for r in range(R)]
    with tc.tile_pool(name="lk_p", bufs=4) as lkp, \
         tc.tile_pool(name="tl_p", bufs=2) as tlp, \
         tc.tile_pool(name="L_p", bufs=4) as Lp, \
         tc.tile_pool(name="LT_p", bufs=2) as LTp, \
         tc.tile_pool(name="bc_p", bufs=2) as bcp, \
         tc.tile_pool(name="crs_p", bufs=2) as crsp, \
         tc.tile_pool(name="wrn_p", bufs=2) as wrnp, \
         tc.tile_pool(name="fb_ps", bufs=2, space="PSUM") as fbps, \
         tc.tile_pool(name="tr_ps", bufs=2, space="PSUM") as trps, \
         tc.tile_pool(name="sm_ps", bufs=1, space="PSUM") as smps:
        for s in range(BS):
            crs_t = crsp.tile([R, N], F32, name="crs_t", tag="crs")
            nc.sync.dma_start(crs_t[:], cr32[4 * s:4 * s + 4, :])
            nwb = bcp.tile([P, N], BF16, name="nwb", tag="nwb")
            nc.sync.dma_start(nwb[:], I["sc_negw"][s:s + 1, :].to_broadcast((P, N)))
            pbc = bcp.tile([P, N], BF16, name="pbc", tag="pbc")
            nc.sync.dma_start(pbc[:], I["sc_prec"][s:s + 1, :].to_broadcast((P, N)))
            LT = LTp.tile([P, NB * N], BF16, name="LT", tag="LT")
            LTv = LT[:].rearrange("p (jb i) -> p jb i", jb=NB)
            fwd_ps = fbps.tile([R, N], F32, name="fwd_ps", tag="fb")
            bwd_ps = fbps.tile([R, N], F32, name="bwd_ps", tag="fb")
            for blk in range(NB):
                lk = lkp.tile([P, N], BF16, name="lk", tag="lk")
                nc.gpsimd.dma_start(lk[:], I["link"][s, blk * 128:(blk + 1) * 128, :])
                csl = wwn_cols[:, blk * BS + s:blk * BS + s + 1]
                osl = omw_cols[:, blk * BS + s:blk * BS + s + 1]
                t_l = tlp.tile([P, N], BF16, name="t_l", tag="tl")
                nc.vector.scalar_tensor_tensor(out=t_l[:], in0=nwb[:], scalar=osl,
                                               in1=lk[:], op0=OP.add, op1=OP.mult)
                Lb = Lp.tile([P, N], BF16, name="Lb", tag="Lb")
                nc.vector.scalar_tensor_tensor(out=Lb[:], in0=pbc[:], scalar=csl,
                                               in1=t_l[:], op0=OP.mult, op1=OP.add)
                dsl = slice(blk * 128, (blk + 1) * 128)
                nc.vector.tensor_mul(Lb[:, dsl], Lb[:, dsl], maskoff[:])
                nc.gpsimd.dma_start(Ou["L"][s, blk * 128:(blk + 1) * 128, :], Lb[:])
                wsl = wrb_s[s][:, blk * R:(blk + 1) * R]
                for ch in range(2):
                    cs = slice(ch * 512, (ch + 1) * 512)
                    nc.tensor.matmul(bwd_ps[:, cs], wsl, Lb[:, cs],
                                     start=(blk == 0), stop=(blk == NB - 1))
                for g in range(2):
                    trp = trps.tile([P, 512], BF16, name="trp", tag="trp")
                    for q in range(4):
                        jb = g * 4 + q
                        nc.tensor.transpose(trp[:, q * 128:(q + 1) * 128],
                                            Lb[:, jb * 128:(jb + 1) * 128], identb[:])
                    nc.scalar.copy(LTv[:, g * 4:(g + 1) * 4, blk * 128:(blk + 1) * 128],
                                   trp[:].rearrange("p (q i) -> p q i", q=4))
            for jb in range(NB):
                wsl = wrb_s[s][:, jb * R:(jb + 1) * R]
                for ch in range(2):
                    cs = slice(ch * 512, (ch + 1) * 512)
                    nc.tensor.matmul(fwd_ps[:, cs], wsl, LTv[:, jb, cs],
                                     start=(jb == 0), stop=(jb == NB - 1))
            # wr_n = pi_b*bwd + pi_c*cr + pi_f*fwd   (4, N) base 0
            wrn_t = wrnp.tile([R, N], F32, name="wrn_t", tag="wt")
            nc.vector.scalar_tensor_tensor(out=wrn_t[:], in0=fwd_ps[:],
                                           scalar=pi_cols[2][:, s:s + 1], in1=crs_t[:],
                                           op0=OP.mult, op1=OP.add)
            wrn = wrnp.tile([R, N], F32, name="wrn", tag="wn")
            nc.vector.scalar_tensor_tensor(out=wrn[:], in0=bwd_ps[:],
                                           scalar=pi_cols[0][:, s:s + 1], in1=wrn_t[:],
                                           op0=OP.mult, op1=OP.add)
            nc.sync.dma_start(Ou["wr_n"][s].rearrange("n r -> r n"), wrn[:])
            # wrn columns (128, NB*R) bf16
            wcol_ps = smps.tile([P, NB * R], F32, name="wcol_ps", tag="wcol")
            for blk in range(NB):
                nc.tensor.transpose(wcol_ps[:, blk * R:(blk + 1) * R],
                                    wrn[:, blk * 128:(blk + 1) * 128], ident[:R, :R])
            wrnc = wrnp.tile([P, NB * R], BF16, name="wrnc", tag="wc")
            nc.vector.tensor_copy(wrnc[:], wcol_ps[:])
            # rv_n = M_n^T @ wr_n  (64, 4)
            rvn_ps = smps.tile([Wd, R], F32, name="rvn_ps", tag="rvn")
            for blk in range(NB):
                nc.tensor.matmul(rvn_ps[:], mnew_s[s][:, blk * Wd:(blk + 1) * Wd],
                                 wrnc[:, blk * R:(blk + 1) * R],
                                 start=(blk == 0), stop=(blk == NB - 1))
            rvn_sb = wrnp.tile([Wd, R], F32, name="rvn_sb", tag="rv")
            nc.scalar.copy(rvn_sb[:], rvn_ps[:])
            nc.sync.dma_start(Ou["rv_n"][s], rvn_sb[:])
            for r in range(R):
                nc.vector.tensor_copy(rvnT[r][:, s:s + 1], rvn_sb[:, r:r + 1])

    # =====================================================================
    # Phase E: y = vu + read_flat_n @ W_read
    # =====================================================================
    with tc.tile_pool(name="pe_ps", bufs=1, space="PSUM") as peps:
        y_ps = peps.tile([O, BS], F32, name="y_ps")
        for r in range(R):
            nc.tensor.matmul(y_ps[:], w_rd[r][:], rvnT[r][:], start=(r == 0), stop=(r == 3))
        y_sb = rows_p.tile([O, BS], F32, name="y_sb")
        nc.vector.tensor_add(y_sb[:], y_ps[:], vu_sb[:])
        nc.sync.dma_start(Ou["y"].rearrange("b o -> o b"), y_sb[:])


# =========================================================================
# Host side
# =========================================================================
_CACHE = {}


def _in_specs():
    return [
        ("x", (BS, V), F32), ("h0", (BS, H), F32), ("c0", (BS, H), F32),
        ("W_ihT", (V + R * Wd, 4 * H), BF16), ("W_hhT", (H, 4 * H), BF16),
        ("b_lstm", (4 * H,), F32),
        ("W_v", (H, O), F32), ("W_xi", (H, XI), F32), ("W_read", (R * Wd, O), F32),
        ("rv", (BS, Wd, R), F32), ("M", (BS, N, Wd), F32),
        ("usage", (BS, N), F32), ("prec", (BS, N), F32),
        ("link", (BS, N, N), F32), ("wr", (BS, N, R), F32), ("ww", (BS, N), F32),
        ("ident", (P, P), F32), ("identb", (P, P), BF16), ("maskoff", (P, P), BF16),
    ]


def _out_specs():
    return [
        ("y", (BS, O)), ("h", (BS, H)), ("c", (BS, H)), ("rv_n", (BS, Wd, R)),
        ("M_n", (BS, N, Wd)), ("u", (BS, N)), ("prec_n", (BS, N)),
        ("L", (BS, N, N)), ("wr_n", (BS, N, R)), ("ww_n", (BS, N)),
    ]


def _build():
    if "nc" in _CACHE:
        return _CACHE["nc"]
    from contextlib import ExitStack
    nc = bacc.Bacc("TRN2", target_bir_lowering=False, debug=False,
                   enable_asserts=False, num_devices=NCORES)
    I, Ou = {}, {}
    for nm, shp, dt in _in_specs():
        I[nm] = nc.dram_tensor(f"in_{nm}", list(shp), dt, kind="ExternalInput").ap()
    for nm, shp in _out_specs():
        Ou[nm] = nc.dram_tensor(f"out_{nm}", list(shp), F32, kind="ExternalOutput").ap()
    I["sc_negw"] = nc.dram_tensor("sc_negw", [BS, N], BF16, kind="Internal").ap()
    I["sc_prec"] = nc.dram_tensor("sc_prec", [BS, N], BF16, kind="Internal").ap()
    with tile.TileContext(nc) as tc:
        with ExitStack() as ctx:
            dnc_core(ctx, tc, I, Ou)
    nc.compile()
    _CACHE["nc"] = (nc, I, Ou)
    return _CACHE["nc"]


def make_in_maps(inputs):
    bf = ml_dtypes.bfloat16
    W_ihT = np.ascontiguousarray(np.asarray(inputs["W_ih"]).T).astype(bf)
    W_hhT = np.ascontiguousarray(np.asarray(inputs["W_hh"]).T).astype(bf)
    ident = np.eye(P, dtype=np.float32)
    identb = np.eye(P, dtype=np.float32).astype(bf)
    maskoff = (1.0 - np.eye(P, dtype=np.float32)).astype(bf)
    in_maps = []
    for c in range(NCORES):
        sl = slice(c * BS, (c + 1) * BS)
        m = {
            "in_x": inputs["x"][sl], "in_h0": inputs["h0"][sl], "in_c0": inputs["c0"][sl],
            "in_W_ihT": W_ihT, "in_W_hhT": W_hhT, "in_b_lstm": inputs["b_lstm"],
            "in_W_v": inputs["W_v"], "in_W_xi": inputs["W_xi"], "in_W_read": inputs["W_read"],
            "in_rv": inputs["rv"][sl], "in_M": inputs["M"][sl],
            "in_usage": inputs["usage"][sl], "in_prec": inputs["prec"][sl],
            "in_link": inputs["link"][sl], "in_wr": inputs["wr"][sl], "in_ww": inputs["ww"][sl],
            "in_ident": ident, "in_identb": identb, "in_maskoff": maskoff,
        }
        in_maps.append({k: np.ascontiguousarray(v) for k, v in m.items()})
    return in_maps


def run_on_hw(inputs, trace=False):
    nc, I, Ou = _build()
    res = run_bass_kernel_spmd(nc, make_in_maps(inputs),
                               core_ids=list(range(NCORES)), trace=trace)
    outs = []
    for nm, shp in _out_specs():
        full = np.concatenate([res.results[c][f"out_{nm}"] for c in range(NCORES)], axis=0)
        outs.append(full)
    return tuple(outs), res


def kernel(**inputs):
    outs, _ = run_on_hw(inputs, trace=False)
    return outs
